# revision 17
# baseline (speedup 1.0000x reference)
"""CosRec-style pairwise-MLP recommender kernel for 8 Trainium2 NeuronCores.

Reference computation (per batch element b, L=32, D=64, FC=100):
    embs   = item_emb[seq_var]                      [B, L, D]
    A      = embs @ Wa^T + b1 (Wa = W1[:, :D])      [B, L, FC]
    Bm     = embs @ Wb^T  (Wb = W1[:, D:])          [B, L, FC]
    h1     = relu(A[:,None,:,:] + Bm[:,:,None,:])   [B, L, L, FC]
    h2     = relu(h1 @ Wf2^T + bf2)                 [B, L, L, FC]
    x      = h2.sum((1, 2))                         [B, FC]
    out[b,t] = b2[item_var[b,t]] + W2[item_var[b,t]] . cat(x[b], user_emb[user_var[b]])

Strategy: data-parallel over batch (64 examples/core).  Hard-won placement
rules for this runtime/HW:
  - GpSimd must never stream bulk elementwise data: it shares an exclusive
    SBUF port with the DVE and poisons both (~10x).  It only issues the
    indirect-DMA gathers here.
  - DVE accum_out / tensor_tensor_reduce are broken; only ScalarE
    activation accum_out works.  So relu2(+bf2)+pair-sum lives on ACT.
  - Standalone LDWEIGHTS is unsupported for fp32/f32r, so every matmul
    pays a weight reload (~200ns+); keep the per-example matmul count at 2
    (layer 2 only) and batch layer 1 over all rows up front.
  - PE row-packed concurrent matmuls accumulating into the same PSUM bank
    hard-fault the device.  Don't.
Main loop per 2 examples: one DVE broadcast-ADD [100, 2048] (outer sum,
fp32 in, fp16 out -- fp16 keeps 10 mantissa bits AND the DVE 16-bit 4x
mode, unlike bf16 which fails the 2e-2 precision gate), one DVE in-place
relu (4x), then per example 2 fp16 matmuls into PSUM and one ACT
relu+bias+accum -> x column.
"""

import os
import sys

import numpy as np

sys.path.insert(0, "/opt/trn_rl_repo")

import concourse.bass as bass
import concourse.tile as tile
from concourse import bacc, mybir
from concourse.bass_utils import run_bass_kernel_spmd
from concourse.masks import make_identity
from contextlib import ExitStack

N_CORES = 8
B_FULL = 512
BPC = B_FULL // N_CORES  # 64 examples per core
L = 32
D = 64
FC = 100
T = 3
NROW = BPC * L           # 2048 gathered rows per core
NTILE = NROW // 128      # 16 gather tiles
NCHUNK = 8               # emission chunks (L1/examples interleaved)
F32 = mybir.dt.float32
F32R = mybir.dt.float32r
BF16 = mybir.dt.bfloat16
F16 = mybir.dt.float16
I32 = mybir.dt.int32

# ---- tunables -------------------------------------------------------------
CFG = dict(
    pre_dt="f32r",     # pre dtype: f32r (PE keeps wide precision), f16, bf16
    l1_f32r=True,      # batch layer-1 matmuls in fp32r
    add_pair=True,     # outer-sum ADD processes 2 examples per DVE instruction
    r1_act_num=2,      # of every 8 example-pair groups, how many run relu1 on ACT
    h2_bufs=3,         # PSUM slots of [100, 1024] (2 banks each)
    pre_bufs=3,
)

_PROG_CACHE = {}


def _build_program(cfg):
    nc = bacc.Bacc()

    seq_idx = nc.dram_tensor("seq_idx", [128, NTILE], I32, kind="ExternalInput")
    user_idx = nc.dram_tensor("user_idx", [BPC, 1], I32, kind="ExternalInput")
    item_idx = nc.dram_tensor("item_idx", [BPC, T], I32, kind="ExternalInput")
    item_emb = nc.dram_tensor("item_emb", [100000, D], F32, kind="ExternalInput")
    user_emb = nc.dram_tensor("user_emb", [100000, D], F32, kind="ExternalInput")
    W2 = nc.dram_tensor("W2", [100000, FC + D], F32, kind="ExternalInput")
    b2 = nc.dram_tensor("b2", [100000, 1], F32, kind="ExternalInput")
    W1 = nc.dram_tensor("W1", [FC, 2 * D], F32, kind="ExternalInput")
    b1 = nc.dram_tensor("b1", [FC, 1], F32, kind="ExternalInput")
    Wf2 = nc.dram_tensor("Wf2", [FC, FC], F32, kind="ExternalInput")
    bf2 = nc.dram_tensor("bf2", [FC, 1], F32, kind="ExternalInput")
    out_d = nc.dram_tensor("out", [BPC, T], F32, kind="ExternalOutput")

    Relu = mybir.ActivationFunctionType.Relu
    Ident = mybir.ActivationFunctionType.Identity
    Add = mybir.AluOpType.add
    Mult = mybir.AluOpType.mult
    Max = mybir.AluOpType.max

    l1dt = F32R if cfg["l1_f32r"] else F32
    predt = {"f16": F16, "bf16": BF16, "f32r": F32R}[cfg["pre_dt"]]
    r1_act = [((i // 2) % 8) < cfg["r1_act_num"] for i in range(BPC)]
    bstep = 2 if cfg["add_pair"] else 1

    with ExitStack() as ctx:
        tc = ctx.enter_context(tile.TileContext(nc))
        const = ctx.enter_context(tc.tile_pool(name="const", bufs=1))
        gat = ctx.enter_context(tc.tile_pool(name="gat", bufs=6))
        prep = ctx.enter_context(tc.tile_pool(name="pre", bufs=cfg["pre_bufs"]))
        scrp = ctx.enter_context(tc.tile_pool(name="scr", bufs=2))
        ps = ctx.enter_context(tc.tile_pool(name="ps", bufs=2, space="PSUM"))
        ps2 = ctx.enter_context(tc.tile_pool(name="ps2", bufs=cfg["h2_bufs"], space="PSUM"))

        # ---------------- constants & weights ----------------
        # idx first: the gathers are the longest prologue chain (GpSimd-serial)
        idx_sb = const.tile([128, NTILE], I32)
        nc.sync.dma_start(out=idx_sb[:], in_=seq_idx[:, :])
        ident = const.tile([128, 128], F32)
        make_identity(nc, ident[:])

        # ---- all embedding gathers issued up front (GpSimd streams them
        # while the PE does the weight transposes) ----
        g_tiles = []
        for t in range(NTILE):
            g = gat.tile([128, D], F32)
            nc.gpsimd.indirect_dma_start(
                out=g[:],
                out_offset=None,
                in_=item_emb[:, :],
                in_offset=bass.IndirectOffsetOnAxis(ap=idx_sb[:, t : t + 1], axis=0),
            )
            g_tiles.append(g)

        w1_sb = const.tile([FC, 2 * D], F32)
        nc.sync.dma_start(out=w1_sb[:], in_=W1[:, :])
        wf2_sb = const.tile([FC, FC], F32)
        nc.sync.dma_start(out=wf2_sb[:], in_=Wf2[:, :])
        b1_sb = const.tile([FC, 1], F32)
        nc.sync.dma_start(out=b1_sb[:], in_=b1[:, :])
        bf2_sb = const.tile([FC, 1], F32)
        nc.sync.dma_start(out=bf2_sb[:], in_=bf2[:, :])
        uidx_sb = const.tile([BPC, 1], I32)
        nc.sync.dma_start(out=uidx_sb[:], in_=user_idx[:, :])
        iidx_sb = const.tile([BPC, T], I32)
        nc.sync.dma_start(out=iidx_sb[:], in_=item_idx[:, :])

        # WaT/WbT: [64, 100] = (W1[:, :D]).T and (W1[:, D:]).T
        waT = const.tile([D, FC], l1dt)
        wbT = const.tile([D, FC], l1dt)
        for half, dst in ((0, waT), (1, wbT)):
            w1h_ps = ps.tile([D, FC], F32, tag="ps")
            nc.tensor.transpose(
                w1h_ps[:], w1_sb[:, half * D : (half + 1) * D], ident[:FC, :FC]
            )
            nc.vector.tensor_copy(dst[:], w1h_ps[:])

        # Wf2T: [100, 100] = Wf2.T  (same dtype as pre for the L2 matmul)
        wf2t_ps = ps.tile([FC, FC], F32, tag="ps")
        nc.tensor.transpose(wf2t_ps[:], wf2_sb[:], ident[:FC, :FC])
        wf2t = const.tile([FC, FC], predt)
        nc.vector.tensor_copy(wf2t[:], wf2t_ps[:])

        embsT = const.tile([D, NROW], l1dt)
        A_t = const.tile([FC, NROW], F32)   # A' = embs@Wa^T + b1   (bias folded)
        Bm_t = const.tile([FC, NROW], F32)  # Bm = embs@Wb^T
        x = const.tile([FC, BPC], F32)      # x[:, b] = sum_{a,c} h2[b, a, c, :]

        # Chunk-interleaved emission: gathers+transposes, batch layer-1, then
        # that chunk's examples — lets the main loop start ~3 chunks early.
        CB = BPC // NCHUNK   # examples per chunk
        CT = NTILE // NCHUNK  # gather tiles per chunk
        CW = NROW // NCHUNK  # rows per chunk
        for chunk in range(NCHUNK):
            # ---- transpose this chunk's gathered tiles (2 per PSUM tile,
            #      one batched DVE copy: fewer per-inst bubbles) ----
            for t0 in range(chunk * CT, (chunk + 1) * CT, 2):
                tp = ps.tile([D, 256], F32, tag="ps")
                nc.tensor.transpose(tp[:, 0:128], g_tiles[t0][:], ident[:, :])
                nc.tensor.transpose(tp[:, 128:256], g_tiles[t0 + 1][:], ident[:, :])
                nc.vector.tensor_copy(embsT[:, t0 * 128 : (t0 + 2) * 128], tp[:])

            # ---- batch layer 1 for this chunk's rows ----
            sl = slice(chunk * CW, (chunk + 1) * CW)
            pa = ps.tile([FC, CW], F32, tag="ps")
            nc.tensor.matmul(pa[:], lhsT=waT[:], rhs=embsT[:, sl], start=True, stop=True)
            nc.scalar.activation(A_t[:, sl], pa[:], Ident, bias=b1_sb[:, 0:1])
            pb = ps.tile([FC, CW], F32, tag="ps")
            nc.tensor.matmul(pb[:], lhsT=wbT[:], rhs=embsT[:, sl], start=True, stop=True)
            nc.vector.tensor_copy(Bm_t[:, sl], pb[:])

            # ---- main loop over this chunk's examples ----
            for b in range(chunk * CB, (chunk + 1) * CB, bstep):
                sl2 = slice(b * L, (b + bstep) * L)
                pre = prep.tile([FC, bstep * L * L], predt)
                # pre[f, b2, a, c] = A'[f, b2*L + c] + Bm[f, b2*L + a]
                in0 = (
                    A_t[:, sl2]
                    .rearrange("p (b2 c) -> p b2 c", b2=bstep)
                    .unsqueeze(2)
                    .to_broadcast([FC, bstep, L, L])
                )
                in1 = (
                    Bm_t[:, sl2]
                    .rearrange("p (b2 a) -> p b2 a", b2=bstep)
                    .unsqueeze(3)
                    .to_broadcast([FC, bstep, L, L])
                )
                nc.vector.tensor_tensor(
                    out=pre[:].rearrange("p (b2 a c) -> p b2 a c", b2=bstep, a=L),
                    in0=in0,
                    in1=in1,
                    op=Add,
                )
                # relu1 in place (bf16 -> 4x DVE mode)
                if r1_act[b]:
                    nc.scalar.activation(pre[:], pre[:], Relu)
                else:
                    nc.vector.tensor_scalar_max(pre[:], pre[:], 0.0)
                for j in range(bstep):
                    bb = b + j
                    # layer 2: h2pre = Wf2T.T @ h1  (PSUM, 2 banks)
                    h2p = ps2.tile([FC, L * L], F32, tag="ps2")
                    for h in range(2):
                        hs = slice(h * 512, (h + 1) * 512)
                        nc.tensor.matmul(
                            h2p[:, hs],
                            lhsT=wf2t[:],
                            rhs=pre[:, j * L * L + h * 512 : j * L * L + (h + 1) * 512],
                            start=True,
                            stop=True,
                        )
                    # relu2(+bf2) with fused pair-sum accumulation -> x[:, bb]
                    h2s = scrp.tile([FC, L * L], F32, tag="h2s")
                    nc.scalar.activation(
                        h2s[:], h2p[:], Relu,
                        bias=bf2_sb[:, 0:1],
                        accum_out=x[:, bb : bb + 1],
                    )

        # ---------------- gathers for the final stage ----------------
        ug = const.tile([BPC, D], F32)
        nc.gpsimd.indirect_dma_start(
            out=ug[:],
            out_offset=None,
            in_=user_emb[:, :],
            in_offset=bass.IndirectOffsetOnAxis(ap=uidx_sb[:, 0:1], axis=0),
        )
        w2g = []
        for t in range(T):
            w2g_t = const.tile([BPC, FC + D], F32, tag=f"w2g{t}")
            nc.gpsimd.indirect_dma_start(
                out=w2g_t[:],
                out_offset=None,
                in_=W2[:, :],
                in_offset=bass.IndirectOffsetOnAxis(ap=iidx_sb[:, t : t + 1], axis=0),
            )
            w2g.append(w2g_t)
        b2g = const.tile([BPC, T], F32)
        for t in range(T):
            nc.gpsimd.indirect_dma_start(
                out=b2g[:, t : t + 1],
                out_offset=None,
                in_=b2[:, :],
                in_offset=bass.IndirectOffsetOnAxis(ap=iidx_sb[:, t : t + 1], axis=0),
            )

        # ---------------- final: out[b, t] = b2 + W2row . cat(x, uemb) ------
        xT_ps = ps.tile([BPC, FC], F32, tag="ps")
        nc.tensor.transpose(xT_ps[:], x[:], ident[:FC, :FC])
        xT = const.tile([BPC, FC], F32)
        nc.vector.tensor_copy(xT[:], xT_ps[:])

        # (DVE accum / tensor_tensor_reduce are broken — use mult + ACT accum)
        out_sb = const.tile([BPC, T], F32)
        for t in range(T):
            scr = scrp.tile([BPC, FC + D], F32, tag="fin")
            nc.vector.tensor_tensor(
                out=scr[:, 0:FC], in0=w2g[t][:, 0:FC], in1=xT[:], op=Mult
            )
            nc.vector.tensor_tensor(
                out=scr[:, FC:], in0=w2g[t][:, FC:], in1=ug[:], op=Mult
            )
            acc = scrp.tile([BPC, 1], F32, tag="facc")
            dummy = scrp.tile([BPC, FC + D], F32, tag="fdum")
            nc.scalar.activation(
                dummy[:], scr[:], Ident,
                accum_out=acc[:],
            )
            nc.vector.tensor_tensor(
                out=out_sb[:, t : t + 1], in0=acc[:], in1=b2g[:, t : t + 1], op=Add
            )
        nc.sync.dma_start(out=out_d[:, :], in_=out_sb[:])

    nc.finalize()
    return nc


def get_program(cfg=None):
    cfg = dict(CFG if cfg is None else cfg)
    key = tuple(sorted(cfg.items()))
    if key not in _PROG_CACHE:
        _PROG_CACHE[key] = _build_program(cfg)
    return _PROG_CACHE[key]


def make_in_maps(inputs):
    """Shard the full-problem inputs into 8 per-core input maps."""
    seq = np.asarray(inputs["seq_var"]).astype(np.int32)
    usr = np.asarray(inputs["user_var"]).astype(np.int32).reshape(B_FULL, 1)
    itm = np.asarray(inputs["item_var"]).astype(np.int32).reshape(B_FULL, T)
    shared = dict(
        item_emb=np.ascontiguousarray(np.asarray(inputs["item_emb"], np.float32)),
        user_emb=np.ascontiguousarray(np.asarray(inputs["user_emb"], np.float32)),
        W2=np.ascontiguousarray(np.asarray(inputs["W2"], np.float32)),
        b2=np.ascontiguousarray(np.asarray(inputs["b2"], np.float32).reshape(-1, 1)),
        W1=np.ascontiguousarray(np.asarray(inputs["W1"], np.float32)),
        b1=np.ascontiguousarray(np.asarray(inputs["b1"], np.float32).reshape(FC, 1)),
        Wf2=np.ascontiguousarray(np.asarray(inputs["Wf2"], np.float32)),
        bf2=np.ascontiguousarray(np.asarray(inputs["bf2"], np.float32).reshape(FC, 1)),
    )
    in_maps = []
    for c in range(N_CORES):
        rows = slice(c * BPC, (c + 1) * BPC)
        flat = seq[rows].reshape(NROW)               # (b*L + l) order
        seq_pm = np.ascontiguousarray(flat.reshape(NTILE, 128).T)  # [128, 16]
        in_maps.append(
            dict(
                shared,
                seq_idx=seq_pm,
                user_idx=np.ascontiguousarray(usr[rows]),
                item_idx=np.ascontiguousarray(itm[rows]),
            )
        )
    return in_maps


def run_sharded(inputs, cfg=None, trace=False, **kwargs):
    nc = get_program(cfg)
    in_maps = make_in_maps(inputs)
    res = run_bass_kernel_spmd(nc, in_maps, list(range(N_CORES)), trace=trace, **kwargs)
    out = np.concatenate([r["out"] for r in res.results], axis=0)
    return out, res


def kernel(**inputs) -> np.ndarray:
    out, _ = run_sharded(inputs)
    return out
